# revision 1
# baseline (speedup 1.0000x reference)
"""DepLabeledGCN Trainium2 kernel — data-parallel variant (no collectives).

Each core processes ITS OWN batch with ALL 48 label matrices:
    s-phase:  sT[l,kc] chunks = per-label masked-adjacency matmuls (fp16,
              masks exact 0/1), label PAIRS fused into N=256 matmuls
    msum:     msg = sum_{l,kc} sT[l,kc] @ W_l^T[kc], 192 accumulating
              matmuls into one PSUM bank per layer
    relu(msg * 1/denom) -> next layer h (one DVE op)
then the 2-layer MLP (PE-transpose + packed PSUM) on the same core.

Weights: 24 MB fp16 streamed label-by-label from HBM; the first R_RES
labels stay SBUF-resident for layer 2, the rest are re-streamed.  No
inter-core communication at all (batch B=8 == 8 cores), which avoids the
~40us ncfw entry barrier + AllReduce/ReduceScatter serialization measured
in the label-sharded variant.
"""

import sys

if '/opt/trn_rl_repo' not in sys.path:
    sys.path.insert(0, '/opt/trn_rl_repo')

import numpy as np

B, N, D, L = 8, 128, 512, 48
NCORES = 8
KC = D // 128
NUM_LAYERS = 2
R_RES = 32              # labels kept resident for layer 2
NP = L // 2             # label pairs per layer

_CACHE = {}


def _build_nc():
    import concourse.bass as bass
    import concourse.mybir as mybir
    import concourse.tile as tile
    from concourse import bacc
    from concourse.masks import make_identity

    dt = mybir.dt
    f32 = dt.float32
    f16 = dt.float16
    Alu = mybir.AluOpType

    nc = bacc.Bacc("TRN2", target_bir_lowering=False, debug=False,
                   num_devices=NCORES)

    gcn_e = nc.dram_tensor("gcn", [N, D], f32, kind="ExternalInput").ap()
    adjT_e = nc.dram_tensor("adjT", [N, N], f32, kind="ExternalInput").ap()
    labT_e = nc.dram_tensor("labT", [N, N], f32, kind="ExternalInput").ap()
    adjR_e = nc.dram_tensor("adjR", [N, N], f32, kind="ExternalInput").ap()
    wT_e = nc.dram_tensor("wT", [128, L, KC, D], f16, kind="ExternalInput").ap()
    w0T_e = nc.dram_tensor("w0T", [128, KC, D], f16, kind="ExternalInput").ap()
    w1T_e = nc.dram_tensor("w1T", [128, KC, D], f16, kind="ExternalInput").ap()
    b0_e = nc.dram_tensor("b0", [128, KC], f32, kind="ExternalInput").ap()
    b1_e = nc.dram_tensor("b1", [128, KC], f32, kind="ExternalInput").ap()
    out_e = nc.dram_tensor("out", [KC, 128, N], f32, kind="ExternalOutput").ap()

    with tile.TileContext(nc) as tc:
        with (
            tc.tile_pool(name="const", bufs=1) as cpool,
            tc.tile_pool(name="sT", bufs=3) as sT_pool,
            tc.tile_pool(name="wst", bufs=5) as wst_pool,
            tc.tile_pool(name="spsum", bufs=3, space="PSUM") as spsum,
            tc.tile_pool(name="mpsum", bufs=2, space="PSUM") as mpsum,
        ):
            # -------- critical-path input loads -----------------------------
            adjT_sb = cpool.tile([128, N], f32, tag="adjT")
            nc.sync.dma_start(adjT_sb[:], adjT_e)
            labT_sb = cpool.tile([128, N], f32, tag="labT")
            nc.sync.dma_start(labT_sb[:], labT_e)
            gcn_sb = cpool.tile([128, D], f32, tag="gcn_sb")
            nc.sync.dma_start(gcn_sb[:], gcn_e)

            h = [cpool.tile([128, D], f16, tag=f"h{ly}", name=f"h{ly}")
                 for ly in range(NUM_LAYERS + 1)]
            nc.scalar.copy(h[0][:], gcn_sb[:])

            # resident weights, loaded per label (just-in-time for layer 1)
            wres = cpool.tile([128, R_RES, KC, D], f16, tag="wres")
            for l in range(R_RES):
                nc.sync.dma_start(wres[:, l], wT_e[:, l])

            # -------- masks: maskT[j, l, i] = (labT == l) * adjT ------------
            # pairs 0..5 upfront; the rest interleaved into the layer-1 loop
            # so the first sT psum->sbuf copies aren't queued behind 48 ops
            maskT = cpool.tile([128, L, N], f16, tag="maskT")

            def emit_mask(l):
                nc.vector.scalar_tensor_tensor(
                    out=maskT[:, l, :],
                    in0=labT_sb[:],
                    scalar=float(l),
                    in1=adjT_sb[:],
                    op0=Alu.is_equal,
                    op1=Alu.mult,
                )

            for l in range(12):
                emit_mask(l)

            # -------- GCN layers --------------------------------------------
            def emit_s(ly, p):
                """s-phase for label pair p: one N=256 matmul per kc."""
                ps = spsum.tile([128, KC, 2, 128], f32, tag="spsum",
                                name="spsum")
                for kc in range(KC):
                    nc.tensor.matmul(
                        ps[:, kc, :, :],
                        lhsT=h[ly][:, kc * 128:(kc + 1) * 128],
                        rhs=maskT[:, 2 * p:2 * p + 2, :],
                        start=True, stop=True,
                    )
                sT = sT_pool.tile([128, 2, KC, 128], f16, tag="sT", name="sT")
                srcp = ps.rearrange("q kc l i -> q l kc i")
                if p % 2 == 0:
                    nc.vector.tensor_copy(sT[:], srcp)
                else:
                    nc.scalar.copy(sT[:], srcp)
                return sT

            def get_w(ly, p):
                """Weight pair p: resident slice or streamed tile."""
                if 2 * p + 1 < R_RES:
                    return wres[:, 2 * p:2 * p + 2]
                w = wst_pool.tile([128, 2, KC, D], f16, tag="wst", name="wst")
                nc.sync.dma_start(w[:], wT_e[:, 2 * p:2 * p + 2])
                return w

            adjR_sb = cpool.tile([128, N], f32, tag="adjR")
            nc.sync.dma_start(adjR_sb[:], adjR_e)
            den = cpool.tile([128, 1], f32, tag="den")
            nc.vector.tensor_reduce(den[:], adjR_sb[:], mybir.AxisListType.X,
                                    Alu.add)
            nc.vector.tensor_scalar_add(den[:], den[:], 1.0)
            recip = cpool.tile([128, 1], f32, tag="recip")
            nc.vector.reciprocal(recip[:], den[:])

            for ly in range(NUM_LAYERS):
                pm = mpsum.tile([128, D], f32, tag="mm", name="mm")
                sT_q = [emit_s(ly, 0), emit_s(ly, 1)]
                for p in range(NP):
                    if ly == 0 and 2 * (p + 6) < L:
                        emit_mask(2 * (p + 6))
                        emit_mask(2 * (p + 6) + 1)
                    if p + 2 < NP:
                        sT_q.append(emit_s(ly, p + 2))
                    w = get_w(ly, p)
                    sT = sT_q[p]
                    for l2 in range(2):
                        for kc in range(KC):
                            i = (p * 2 + l2) * KC + kc
                            nc.tensor.matmul(
                                pm[:],
                                lhsT=sT[:, l2, kc, :],
                                rhs=w[:, l2, kc, :],
                                start=(i == 0), stop=(i == L * KC - 1),
                            )
                # relu(msg * recip) -> next h (fp16)
                nc.vector.tensor_scalar(h[ly + 1][:], pm[:], recip[:], 0.0,
                                        Alu.mult, Alu.max)

            # -------- MLP ---------------------------------------------------
            b0_sb = cpool.tile([128, KC], f32, tag="b0")
            nc.sync.dma_start(b0_sb[:], b0_e)
            b1_sb = cpool.tile([128, KC], f32, tag="b1")
            nc.sync.dma_start(b1_sb[:], b1_e)
            w0T_sb = cpool.tile([128, KC, D], f16, tag="w0T")
            nc.sync.dma_start(w0T_sb[:], w0T_e)
            w1T_sb = cpool.tile([128, KC, D], f16, tag="w1T")
            nc.sync.dma_start(w1T_sb[:], w1T_e)
            h_own = h[NUM_LAYERS]
            identity = cpool.tile([128, 128], f16, tag="ident")
            make_identity(nc, identity[:])
            hT = cpool.tile([128, KC, 128], f16, tag="hT")
            pt = mpsum.tile([128, KC, 128], f16, tag="mm", name="ptr")
            for kc in range(KC):
                nc.tensor.transpose(pt[:, kc, :],
                                    h_own[:, kc * 128:(kc + 1) * 128],
                                    identity[:])
            nc.vector.tensor_copy(hT[:], pt[:])

            x1T = cpool.tile([128, KC, 128], f16, tag="x1T")
            px1 = mpsum.tile([128, KC, 128], f32, tag="mm", name="px1")
            for blk in range(KC):
                for kc in range(KC):
                    nc.tensor.matmul(
                        px1[:, blk, :],
                        lhsT=w0T_sb[:, kc, blk * 128:(blk + 1) * 128],
                        rhs=hT[:, kc, :],
                        start=(kc == 0), stop=(kc == KC - 1),
                    )
            for blk in range(KC):
                nc.vector.tensor_scalar(x1T[:, blk, :], px1[:, blk, :],
                                        b0_sb[:, blk:blk + 1], 0.0,
                                        Alu.add, Alu.max)

            x2 = cpool.tile([128, KC, 128], f32, tag="x2")
            px2 = mpsum.tile([128, KC, 128], f32, tag="mm", name="px2")
            for blk in range(KC):
                for kc in range(KC):
                    nc.tensor.matmul(
                        px2[:, blk, :],
                        lhsT=w1T_sb[:, kc, blk * 128:(blk + 1) * 128],
                        rhs=x1T[:, kc, :],
                        start=(kc == 0), stop=(kc == KC - 1),
                    )
            for blk in range(KC):
                nc.vector.tensor_scalar(x2[:, blk, :], px2[:, blk, :],
                                        b1_sb[:, blk:blk + 1], 0.0,
                                        Alu.add, Alu.max)

            for blk in range(KC):
                nc.sync.dma_start(out_e[blk], x2[:, blk, :])

    nc.compile()
    return nc


def _get_nc():
    if "nc" not in _CACHE:
        _CACHE["nc"] = _build_nc()
    return _CACHE["nc"]


def kernel(gcn_inputs, word_seq_len, adj_matrix, dep_label_matrix,
           w_params, mlp_w0, mlp_b0, mlp_w1, mlp_b1, **_unused):
    from concourse.bass_utils import run_bass_kernel_spmd

    gcn = np.asarray(gcn_inputs, dtype=np.float32)
    adj = np.asarray(adj_matrix, dtype=np.float32)
    lab = np.asarray(dep_label_matrix)
    w = np.asarray(w_params, dtype=np.float32)
    w0 = np.asarray(mlp_w0, dtype=np.float32)
    w1 = np.asarray(mlp_w1, dtype=np.float32)
    b0 = np.asarray(mlp_b0, dtype=np.float32)
    b1 = np.asarray(mlp_b1, dtype=np.float32)

    # wT[kmod, l, kc, d] = w[l, d, kc*128+kmod]  (shared by all cores)
    wT = w.transpose(0, 2, 1).reshape(L, KC, 128, D).transpose(2, 0, 1, 3)
    wT = np.ascontiguousarray(wT).astype(np.float16)
    w0T = np.ascontiguousarray(
        w0.T.reshape(KC, 128, D).transpose(1, 0, 2)).astype(np.float16)
    w1T = np.ascontiguousarray(
        w1.T.reshape(KC, 128, D).transpose(1, 0, 2)).astype(np.float16)
    b0r = np.ascontiguousarray(b0.reshape(KC, 128).T)
    b1r = np.ascontiguousarray(b1.reshape(KC, 128).T)
    labf = lab.astype(np.float32)

    in_maps = []
    for c in range(NCORES):
        in_maps.append({
            "gcn": gcn[c],
            "adjT": np.ascontiguousarray(adj[c].T),
            "labT": np.ascontiguousarray(labf[c].T),
            "adjR": np.ascontiguousarray(adj[c]),
            "wT": wT,
            "w0T": w0T,
            "w1T": w1T,
            "b0": b0r,
            "b1": b1r,
        })

    nc = _get_nc()
    res = run_bass_kernel_spmd(nc, in_maps, list(range(NCORES)))

    out = np.empty((B, N, D), dtype=np.float32)
    for c in range(NCORES):
        arr = res.results[c]["out"]          # [dblk, dmod, i]
        out[c] = np.transpose(arr, (2, 0, 1)).reshape(N, D)
    return out

